# revision 18
# baseline (speedup 1.0000x reference)
"""Bidirectional masked LSTM (Keras semantics) on 8 Trainium2 NeuronCores.

Problem: B=128, T=512, D=3, UNITS=512, G=2048 per direction.
Sharding: 8 cores = 2 directions x 4 batch quarters (B=32/core), no cross-core
communication. Backward direction runs on time-reversed inputs prepared on host.

Per-core per-step device computation (z fold layout):
  - 4 PE col-groups (tile_position=(0,32g)): group g owns unit-chunk g
    (units 128g..128g+128). PSUM z tile [128, 512]: partition 32g+i =
    (unit-chunk g, batch i); free = [i|f|o|g] gate blocks of 128 (unit within
    chunk). Per group: 1 K=5 matmul (x, bias, mask-fold) + 4 K=128 matmuls
    (h.T stationary, Wh moving) accumulate z.
  - Mask folded into gates: row 4 of the x-side stationary is (1-m_t) with
    weight row (-BIG for i, +BIG for f, -BIG for o, 0 for g), so masked steps
    give i'=o'=0, f'=1 => c carries, and h = h_prev*(1-m) + o'*tanh(c).
  - EW: sigmoid on z[:,0:384], tanh on z[:,384:512]; c = f'*c + i'*g;
    h = (h_prev * (1-m_t)) + o'*tanh(c) via scalar_tensor_tensor.
  - PE transpose of h fold -> h.T stationary for next step.
"""

import sys

sys.path.insert(0, "/opt/trn_rl_repo")

import ml_dtypes
import numpy as np

import concourse.bacc as bacc
import concourse.bass as bass
import concourse.mybir as mybir
import concourse.tile as tile
from concourse.bass_utils import run_bass_kernel_spmd
from concourse.masks import make_identity

B, T, D, U = 128, 512, 3, 512
G = 4 * U
NCORES = 8
BQ = B // 4            # batch per core (32)
NG = 4                 # PE col groups
KT = U // 128          # K tiles (4)
BIG = 50.0             # mask logit fold
NDUMMY = 11            # PE filler matmuls per step (HAM warmth)
TRACE = False          # set True (with axon_prof imported) to profile
F32 = mybir.dt.float32
F32R = mybir.dt.float32r
BF16 = mybir.dt.float16  # 16-bit tensors use fp16: 10-bit mantissa, values all in [-60, 60]

# gate order within each group's 512 free columns: [f | i | o | g]
_SRC_GATE_OFF = [512, 0, 1536, 1024]  # Keras z layout: i,f,g,o


def _group_cols():
    """cols[g, c] = source Wh/Wx column for group g, free column c."""
    cols = np.empty((NG, 512), np.int64)
    ar = np.arange(128)
    for g in range(NG):
        for gi in range(4):
            cols[g, 128 * gi:128 * gi + 128] = _SRC_GATE_OFF[gi] + 128 * g + ar
    return cols


_COLS = _group_cols()


def _build_program(trace_label=None):
    nc = bacc.Bacc("TRN2", target_bir_lowering=False, debug=False,
                   enable_asserts=False, num_devices=NCORES)

    whc_d = nc.dram_tensor("whc", [128, KT * NG * 512], BF16, kind="ExternalInput").ap()
    wxc_d = nc.dram_tensor("wxc", [5, NG * 512], BF16, kind="ExternalInput").ap()
    xaug_d = nc.dram_tensor("xaug", [5, T * BQ], BF16, kind="ExternalInput").ap()
    minv_d = nc.dram_tensor("minv", [128, T], F32, kind="ExternalInput").ap()

    ys_d = nc.dram_tensor("ys", [T, 128, 128], BF16, kind="ExternalOutput").ap()
    hfin_d = nc.dram_tensor("hfin", [128, 128], BF16, kind="ExternalOutput").ap()
    cfin_d = nc.dram_tensor("cfin", [128, 128], F32, kind="ExternalOutput").ap()

    with tile.TileContext(nc) as tc:
        with (
            tc.tile_pool(name="const", bufs=1) as cpool,
            tc.tile_pool(name="state", bufs=3) as spool,
            tc.tile_pool(name="work", bufs=3) as wpool,
            tc.tile_pool(name="zps", bufs=3, space="PSUM") as zpool,
            tc.tile_pool(name="trps", bufs=2, space="PSUM") as tpool,
            tc.tile_pool(name="wps", bufs=1, space="PSUM") as wupool,
        ):
            whc = cpool.tile([128, KT * NG * 512], BF16)
            wxc = cpool.tile([5, NG * 512], BF16)
            xaug = cpool.tile([5, T * BQ], BF16)
            minv = cpool.tile([128, T], F32)
            ident = cpool.tile([128, 128], BF16)

            nc.sync.dma_start(whc[:], whc_d[:])
            nc.sync.dma_start(wxc[:], wxc_d[:])
            nc.sync.dma_start(xaug[:], xaug_d[:])
            nc.sync.dma_start(minv[:], minv_d[:])
            make_identity(nc, ident[:])

            h_prev = spool.tile([128, 128], BF16, tag="h", bufs=5)
            c_prev = spool.tile([128, 128], F32, tag="c")
            hT = spool.tile([128, 128], BF16, tag="hT", bufs=4)
            nc.vector.memset(h_prev[:], 0.0)
            nc.vector.memset(c_prev[:], 0.0)
            nc.vector.memset(hT[:], 0.0)

            # PE warmup: ~20 back-to-back matmuls (~8us) flips the HAM clock
            # gate to 2.4 GHz; steady-state PE gaps stay under the ~3.4us MID
            # window so it never re-throttles.
            wups = wupool.tile([128, 512], F32)
            for _ in range(20):
                nc.tensor.matmul(wups[:], whc[:, 0:128], whc[:, 0:512],
                                 start=True, stop=True)

            for t in range(T):
                z = zpool.tile([128, 512], F32, tag="z")
                # x-side matmuls first: no dependency on h, can run during
                # previous step's elementwise chain.
                xs = xaug[:, t * BQ:(t + 1) * BQ]
                for g in range(NG):
                    nc.tensor.matmul(
                        z[32 * g:32 * g + 32, :], xs,
                        wxc[:, g * 512:(g + 1) * 512],
                        start=True, stop=False, tile_position=(0, 32 * g),
                    )
                for k in range(KT):
                    hs = hT[:, 32 * k:32 * k + 32]
                    for g in range(NG):
                        nc.tensor.matmul(
                            z[32 * g:32 * g + 32, :], hs,
                            whc[:, (k * NG + g) * 512:(k * NG + g + 1) * 512],
                            start=False, stop=(k == KT - 1),
                            tile_position=(0, 32 * g),
                        )

                # filler matmuls: keep the PE HAM-busy through the elementwise
                # window so the clock gate stays at 2.4 GHz (idle re-throttles
                # to 1.2 GHz and the per-step bursts never re-warm it)
                for _ in range(NDUMMY):
                    nc.tensor.matmul(wups[:], whc[:, 0:128], whc[:, 0:512],
                                     start=True, stop=True)

                # mask-blend term: independent of z, runs during the MM phase
                hm = wpool.tile([128, 128], BF16, tag="hm")
                nc.vector.tensor_scalar_mul(hm[:], h_prev[:], minv[:, t:t + 1])

                gb = wpool.tile([128, 128], BF16, tag="gb")
                sbif = wpool.tile([128, 256], BF16, tag="sbif")
                sbo = wpool.tile([128, 128], BF16, tag="sbo")
                nc.scalar.activation(sbif[:], z[:, 0:256],
                                     mybir.ActivationFunctionType.Sigmoid)
                nc.scalar.activation(gb[:], z[:, 384:512],
                                     mybir.ActivationFunctionType.Tanh)
                nc.scalar.activation(sbo[:], z[:, 256:384],
                                     mybir.ActivationFunctionType.Sigmoid)

                ab = wpool.tile([128, 128], BF16, tag="ab")
                cf = wpool.tile([128, 128], F32, tag="cf")
                c_new = spool.tile([128, 128], F32, tag="c")
                nc.vector.tensor_mul(cf[:], sbif[:, 0:128], c_prev[:])
                nc.vector.tensor_mul(ab[:], sbif[:, 128:256], gb[:])
                nc.vector.tensor_add(c_new[:], cf[:], ab[:])

                th = wpool.tile([128, 128], BF16, tag="th")
                nc.scalar.activation(th[:], c_new[:],
                                     mybir.ActivationFunctionType.Tanh)

                ot = wpool.tile([128, 128], BF16, tag="ot")
                h_new = spool.tile([128, 128], BF16, tag="h", bufs=5)
                nc.vector.tensor_mul(ot[:], sbo[:], th[:])
                nc.vector.tensor_add(h_new[:], hm[:], ot[:])

                tr = tpool.tile([128, 128], BF16, tag="tr")
                hT = spool.tile([128, 128], BF16, tag="hT", bufs=4)
                nc.tensor.transpose(tr[:], h_new[:], ident[:])
                nc.vector.tensor_copy(hT[:], tr[:])

                nc.sync.dma_start(ys_d[t], hT[:])

                h_prev, c_prev = h_new, c_new

            nc.sync.dma_start(hfin_d[:], h_prev[:])
            nc.sync.dma_start(cfin_d[:], c_prev[:])

    nc.compile()
    return nc


_PROGRAM = None


def _program():
    global _PROGRAM
    if _PROGRAM is None:
        _PROGRAM = _build_program()
    return _PROGRAM


def _prep_core(seq, msk, Wx, Wh, b):
    """Per-core input arrays. seq [BQ,T,D] f32, msk [BQ,T] bool (already
    time-reversed for backward cores)."""
    whc = np.empty((128, KT * NG * 512), np.float16)
    for k in range(KT):
        wk = Wh[128 * k:128 * (k + 1)]
        for g in range(NG):
            whc[:, (k * NG + g) * 512:(k * NG + g + 1) * 512] = wk[:, _COLS[g]]

    mrow = np.zeros(G, np.float32)
    mrow[0:512] = -BIG          # i
    mrow[512:1024] = BIG        # f
    mrow[1536:2048] = -BIG      # o
    wxc = np.empty((5, NG * 512), np.float16)
    for g in range(NG):
        blk = slice(g * 512, (g + 1) * 512)
        wxc[0:3, blk] = Wx[:, _COLS[g]]
        wxc[3, blk] = b[_COLS[g]]
        wxc[4, blk] = mrow[_COLS[g]]

    inv = (1.0 - msk.astype(np.float32))            # [BQ, T]
    xaug = np.empty((5, T * BQ), np.float16)
    xaug[0:3] = seq.transpose(2, 1, 0).reshape(3, T * BQ)   # [d, t, i]
    xaug[3] = 1.0
    xaug[4] = inv.T.reshape(T * BQ)                  # [t, i]
    minv = np.tile(inv[None], (4, 1, 1)).reshape(128, T)    # [32j+i, t]
    return {"whc": whc, "wxc": wxc, "xaug": xaug, "minv": minv}


def _unfold_seq(ys):
    """[T,128,128] transposed fold (ys[t, u, 32j+i] = h[i, 128j+u]) -> [BQ, T, U]."""
    a = np.asarray(ys, np.float32).reshape(T, 128, 4, BQ)
    return a.transpose(3, 0, 2, 1).reshape(BQ, T, U)


def _unfold_state(s):
    """[128,128] fold -> [BQ, U]."""
    return np.asarray(s, np.float32).reshape(4, BQ, 128).transpose(1, 0, 2).reshape(BQ, U)


def kernel(sequence, mask, Wx_f, Wh_f, b_f, Wx_b, Wh_b, b_b, W1, b1, **extra):
    sequence = np.asarray(sequence, np.float32)
    mask = np.asarray(mask)
    nc = _program()

    in_maps = []
    for core in range(NCORES):
        d, q = divmod(core, 4)
        sl = slice(q * BQ, (q + 1) * BQ)
        seq, msk = sequence[sl], mask[sl]
        if d == 1:
            seq, msk = seq[:, ::-1], msk[:, ::-1]
        if d == 0:
            in_maps.append(_prep_core(seq, msk, np.asarray(Wx_f), np.asarray(Wh_f), np.asarray(b_f)))
        else:
            in_maps.append(_prep_core(seq, msk, np.asarray(Wx_b), np.asarray(Wh_b), np.asarray(b_b)))

    res = run_bass_kernel_spmd(nc, in_maps, list(range(NCORES)), trace=TRACE)
    if TRACE and res.exec_time_ns:
        print(f"HW exec time: {res.exec_time_ns} ns")

    out_f = np.empty((B, T, U), np.float32)
    out_b = np.empty((B, T, U), np.float32)
    h_f = np.empty((B, U), np.float32)
    h_b = np.empty((B, U), np.float32)
    c_f = np.empty((B, U), np.float32)
    c_b = np.empty((B, U), np.float32)
    for core in range(NCORES):
        d, q = divmod(core, 4)
        sl = slice(q * BQ, (q + 1) * BQ)
        r = res.results[core]
        ys = _unfold_seq(r["ys"])
        if d == 0:
            out_f[sl] = ys
            h_f[sl] = _unfold_state(r["hfin"])
            c_f[sl] = _unfold_state(r["cfin"])
        else:
            out_b[sl] = ys[:, ::-1]
            h_b[sl] = _unfold_state(r["hfin"])
            c_b[sl] = _unfold_state(r["cfin"])

    output = np.concatenate([out_f, out_b], axis=-1)
    state_h = np.stack([h_f, h_b], axis=1)
    state_c = np.stack([c_f, c_b], axis=1)
    pos_agg = (state_h @ np.asarray(W1) + np.asarray(b1)).reshape(B, 1, 8)
    return output, state_h, state_c, pos_agg


# revision 24
# speedup vs baseline: 1.0026x; 1.0026x over previous
"""Bidirectional masked LSTM (Keras semantics) on 8 Trainium2 NeuronCores.

Problem: B=128, T=512, D=3, UNITS=512, G=2048 per direction.
Sharding: 8 cores = 2 directions x 4 batch quarters (B=32/core), no cross-core
communication. Backward direction runs on time-reversed inputs prepared on host.

Per-core per-step device computation (z fold layout, all matmuls fp16 with
fp32 PSUM accumulation):
  - 4 PE col-groups (tile_position=(0,32g)): group g owns unit-chunk g
    (units 128g..128g+128). PSUM z tile [128, 512]: partition 32g+i =
    (unit-chunk g, batch i); free = [f|i|o|g] gate blocks of 128 (unit within
    chunk). Per group: 1 K=5 matmul (x, bias, mask-fold) + 4 K=128 matmuls
    (h.T stationary, Wh moving) accumulate z; the last K tile is split into
    two N=256 halves so the f,i half ungates sigmoid earlier.
  - Mask folded into gates: row 4 of the x-side stationary is (1-m_t) with
    weight row (+BIG for f, -BIG for i/o, 0 for g), so masked steps give
    i'=o'=0, f'=1 => c carries, and h = h_prev*(1-m) + o'*tanh(c).
  - EW: sigmoid(f,i) then tanh(g) then sigmoid(o) on ScalarE; DVE chain
    cf = f'*c, ab = i'*g, c = cf+ab, ot = o'*tanh(c), h = hm + ot where
    hm = h_prev*(1-m_t) is computed off the critical path.
  - PE transpose of h fold -> h.T stationary for next step; dummy filler
    matmuls keep the PE clock gate (HAM) at 2.4 GHz through the elementwise
    window.
"""

import sys

sys.path.insert(0, "/opt/trn_rl_repo")

import ml_dtypes
import numpy as np

import concourse.bacc as bacc
import concourse.bass as bass
import concourse.mybir as mybir
import concourse.tile as tile
from concourse.bass_utils import run_bass_kernel_spmd
from concourse.masks import make_identity

B, T, D, U = 128, 512, 3, 512
G = 4 * U
NCORES = 8
BQ = B // 4            # batch per core (32)
NG = 4                 # PE col groups
KT = U // 128          # K tiles (4)
BIG = 50.0             # mask logit fold
NDUMMY = 10            # PE filler matmuls per step (HAM warmth)
TRACE = False          # set True (with axon_prof imported) to profile
F32 = mybir.dt.float32
F32R = mybir.dt.float32r
BF16 = mybir.dt.float16  # 16-bit tensors use fp16: 10-bit mantissa, values all in [-60, 60]

# gate order within each group's 512 free columns: [f | i | o | g]
_SRC_GATE_OFF = [512, 0, 1536, 1024]  # Keras z layout: i,f,g,o


def _group_cols():
    """cols[g, c] = source Wh/Wx column for group g, free column c."""
    cols = np.empty((NG, 512), np.int64)
    ar = np.arange(128)
    for g in range(NG):
        for gi in range(4):
            cols[g, 128 * gi:128 * gi + 128] = _SRC_GATE_OFF[gi] + 128 * g + ar
    return cols


_COLS = _group_cols()


def _build_program(trace_label=None):
    nc = bacc.Bacc("TRN2", target_bir_lowering=False, debug=False,
                   enable_asserts=False, num_devices=NCORES)

    whc_d = nc.dram_tensor("whc", [128, KT * NG * 512], BF16, kind="ExternalInput").ap()
    wxc_d = nc.dram_tensor("wxc", [5, NG * 512], BF16, kind="ExternalInput").ap()
    xaug_d = nc.dram_tensor("xaug", [5, T * BQ], BF16, kind="ExternalInput").ap()
    minv_d = nc.dram_tensor("minv", [128, T], F32, kind="ExternalInput").ap()

    ys_d = nc.dram_tensor("ys", [T, 128, 128], BF16, kind="ExternalOutput").ap()
    hfin_d = nc.dram_tensor("hfin", [128, 128], BF16, kind="ExternalOutput").ap()
    cfin_d = nc.dram_tensor("cfin", [128, 128], F32, kind="ExternalOutput").ap()

    with tile.TileContext(nc) as tc:
        with (
            tc.tile_pool(name="const", bufs=1) as cpool,
            tc.tile_pool(name="state", bufs=3) as spool,
            tc.tile_pool(name="work", bufs=3) as wpool,
            tc.tile_pool(name="zps", bufs=3, space="PSUM") as zpool,
            tc.tile_pool(name="trps", bufs=2, space="PSUM") as tpool,
            tc.tile_pool(name="wps", bufs=1, space="PSUM") as wupool,
        ):
            whc = cpool.tile([128, KT * NG * 512], BF16)
            wxc = cpool.tile([5, NG * 512], BF16)
            xaug = cpool.tile([5, T * BQ], BF16)
            minv = cpool.tile([128, T], F32)
            ident = cpool.tile([128, 128], BF16)

            nc.sync.dma_start(whc[:], whc_d[:])
            nc.sync.dma_start(wxc[:], wxc_d[:])
            nc.sync.dma_start(xaug[:], xaug_d[:])
            nc.sync.dma_start(minv[:], minv_d[:])
            make_identity(nc, ident[:])

            h_prev = spool.tile([128, 128], BF16, tag="h", bufs=5)
            c_prev = spool.tile([128, 128], F32, tag="c")
            hT = spool.tile([128, 128], BF16, tag="hT", bufs=4)
            nc.vector.memset(h_prev[:], 0.0)
            nc.vector.memset(c_prev[:], 0.0)
            nc.vector.memset(hT[:], 0.0)

            # PE warmup: ~20 back-to-back matmuls (~8us) flips the HAM clock
            # gate to 2.4 GHz; steady-state PE gaps stay under the ~3.4us MID
            # window so it never re-throttles.
            wups = wupool.tile([128, 512], F32)
            for _ in range(20):
                nc.tensor.matmul(wups[:], whc[:, 0:128], whc[:, 0:512],
                                 start=True, stop=True)

            for t in range(T):
                z = zpool.tile([128, 512], F32, tag="z")
                # x-side matmuls first: no dependency on h, can run during
                # previous step's elementwise chain.
                xs = xaug[:, t * BQ:(t + 1) * BQ]
                for g in range(NG):
                    nc.tensor.matmul(
                        z[32 * g:32 * g + 32, :], xs,
                        wxc[:, g * 512:(g + 1) * 512],
                        start=True, stop=False, tile_position=(0, 32 * g),
                    )
                for k in range(KT - 1):
                    hs = hT[:, 32 * k:32 * k + 32]
                    for g in range(NG):
                        nc.tensor.matmul(
                            z[32 * g:32 * g + 32, :], hs,
                            whc[:, (k * NG + g) * 512:(k * NG + g + 1) * 512],
                            start=False, stop=False,
                            tile_position=(0, 32 * g),
                        )
                # last K tile split in half along free dim: the f,i half
                # completes first and ungates sigmoid earlier
                k = KT - 1
                hs = hT[:, 32 * k:32 * k + 32]
                for half in range(2):
                    fs = slice(half * 256, half * 256 + 256)
                    for g in range(NG):
                        nc.tensor.matmul(
                            z[32 * g:32 * g + 32, fs], hs,
                            whc[:, (k * NG + g) * 512 + half * 256:
                                (k * NG + g) * 512 + half * 256 + 256],
                            start=False, stop=True,
                            tile_position=(0, 32 * g),
                        )

                # filler matmuls: keep the PE HAM-busy through the elementwise
                # window so the clock gate stays at 2.4 GHz (idle re-throttles
                # to 1.2 GHz and the per-step bursts never re-warm it)
                for _ in range(NDUMMY):
                    nc.tensor.matmul(wups[:], whc[:, 0:128], whc[:, 0:512],
                                     start=True, stop=True)

                # mask-blend term: independent of z; low priority so the
                # scheduler cannot slot it between transpose and the hT copies
                hm = wpool.tile([128, 128], BF16, tag="hm")
                with tc.high_priority(offset=-20):
                    nc.vector.tensor_scalar_mul(hm[:], h_prev[:], minv[:, t:t + 1])

                gb = wpool.tile([128, 128], BF16, tag="gb")
                sbif = wpool.tile([128, 256], BF16, tag="sbif")
                sbo = wpool.tile([128, 128], BF16, tag="sbo")
                nc.scalar.activation(sbif[:], z[:, 0:256],
                                     mybir.ActivationFunctionType.Sigmoid)
                nc.scalar.activation(gb[:], z[:, 384:512],
                                     mybir.ActivationFunctionType.Tanh)
                nc.scalar.activation(sbo[:], z[:, 256:384],
                                     mybir.ActivationFunctionType.Sigmoid)

                ab = wpool.tile([128, 128], BF16, tag="ab")
                cf = wpool.tile([128, 128], F32, tag="cf")
                c_new = spool.tile([128, 128], F32, tag="c")
                nc.vector.tensor_mul(cf[:], sbif[:, 0:128], c_prev[:])
                nc.vector.tensor_mul(ab[:], sbif[:, 128:256], gb[:])
                nc.vector.tensor_add(c_new[:], cf[:], ab[:])

                th = wpool.tile([128, 128], BF16, tag="th")
                nc.scalar.activation(th[:], c_new[:],
                                     mybir.ActivationFunctionType.Tanh)

                ot = wpool.tile([128, 128], BF16, tag="ot")
                h_new = spool.tile([128, 128], BF16, tag="h", bufs=5)
                nc.vector.tensor_mul(ot[:], sbo[:], th[:])
                nc.vector.tensor_add(h_new[:], hm[:], ot[:])

                tr = tpool.tile([128, 128], BF16, tag="tr")
                hT = spool.tile([128, 128], BF16, tag="hT", bufs=4)
                nc.tensor.transpose(tr[:], h_new[:], ident[:])
                nc.vector.tensor_copy(hT[:, 0:32], tr[:, 0:32])
                nc.vector.tensor_copy(hT[:, 32:128], tr[:, 32:128])

                nc.sync.dma_start(ys_d[t], hT[:])

                h_prev, c_prev = h_new, c_new

            nc.sync.dma_start(hfin_d[:], h_prev[:])
            nc.sync.dma_start(cfin_d[:], c_prev[:])

    nc.compile()
    return nc


_PROGRAM = None


def _program():
    global _PROGRAM
    if _PROGRAM is None:
        _PROGRAM = _build_program()
    return _PROGRAM


def _prep_core(seq, msk, Wx, Wh, b):
    """Per-core input arrays. seq [BQ,T,D] f32, msk [BQ,T] bool (already
    time-reversed for backward cores)."""
    whc = np.empty((128, KT * NG * 512), np.float16)
    for k in range(KT):
        wk = Wh[128 * k:128 * (k + 1)]
        for g in range(NG):
            whc[:, (k * NG + g) * 512:(k * NG + g + 1) * 512] = wk[:, _COLS[g]]

    mrow = np.zeros(G, np.float32)
    mrow[0:512] = -BIG          # i
    mrow[512:1024] = BIG        # f
    mrow[1536:2048] = -BIG      # o
    wxc = np.empty((5, NG * 512), np.float16)
    for g in range(NG):
        blk = slice(g * 512, (g + 1) * 512)
        wxc[0:3, blk] = Wx[:, _COLS[g]]
        wxc[3, blk] = b[_COLS[g]]
        wxc[4, blk] = mrow[_COLS[g]]

    inv = (1.0 - msk.astype(np.float32))            # [BQ, T]
    xaug = np.empty((5, T * BQ), np.float16)
    xaug[0:3] = seq.transpose(2, 1, 0).reshape(3, T * BQ)   # [d, t, i]
    xaug[3] = 1.0
    xaug[4] = inv.T.reshape(T * BQ)                  # [t, i]
    minv = np.tile(inv[None], (4, 1, 1)).reshape(128, T)    # [32j+i, t]
    return {"whc": whc, "wxc": wxc, "xaug": xaug, "minv": minv}


def _unfold_seq(ys):
    """[T,128,128] transposed fold (ys[t, u, 32j+i] = h[i, 128j+u]) -> [BQ, T, U]."""
    a = np.asarray(ys, np.float32).reshape(T, 128, 4, BQ)
    return a.transpose(3, 0, 2, 1).reshape(BQ, T, U)


def _unfold_state(s):
    """[128,128] fold -> [BQ, U]."""
    return np.asarray(s, np.float32).reshape(4, BQ, 128).transpose(1, 0, 2).reshape(BQ, U)


def kernel(sequence, mask, Wx_f, Wh_f, b_f, Wx_b, Wh_b, b_b, W1, b1, **extra):
    sequence = np.asarray(sequence, np.float32)
    mask = np.asarray(mask)
    nc = _program()

    in_maps = []
    for core in range(NCORES):
        d, q = divmod(core, 4)
        sl = slice(q * BQ, (q + 1) * BQ)
        seq, msk = sequence[sl], mask[sl]
        if d == 1:
            seq, msk = seq[:, ::-1], msk[:, ::-1]
        if d == 0:
            in_maps.append(_prep_core(seq, msk, np.asarray(Wx_f), np.asarray(Wh_f), np.asarray(b_f)))
        else:
            in_maps.append(_prep_core(seq, msk, np.asarray(Wx_b), np.asarray(Wh_b), np.asarray(b_b)))

    res = None
    for attempt in range(3):
        try:
            res = run_bass_kernel_spmd(nc, in_maps, list(range(NCORES)), trace=TRACE)
            break
        except Exception:
            # transient NRT_EXEC_UNIT_UNRECOVERABLE device wedges recover on
            # re-execution; re-raise only if persistent
            if attempt == 2:
                raise
    if TRACE and res.exec_time_ns:
        print(f"HW exec time: {res.exec_time_ns} ns")

    out_f = np.empty((B, T, U), np.float32)
    out_b = np.empty((B, T, U), np.float32)
    h_f = np.empty((B, U), np.float32)
    h_b = np.empty((B, U), np.float32)
    c_f = np.empty((B, U), np.float32)
    c_b = np.empty((B, U), np.float32)
    for core in range(NCORES):
        d, q = divmod(core, 4)
        sl = slice(q * BQ, (q + 1) * BQ)
        r = res.results[core]
        ys = _unfold_seq(r["ys"])
        if d == 0:
            out_f[sl] = ys
            h_f[sl] = _unfold_state(r["hfin"])
            c_f[sl] = _unfold_state(r["cfin"])
        else:
            out_b[sl] = ys[:, ::-1]
            h_b[sl] = _unfold_state(r["hfin"])
            c_b[sl] = _unfold_state(r["cfin"])

    output = np.concatenate([out_f, out_b], axis=-1)
    state_h = np.stack([h_f, h_b], axis=1)
    state_c = np.stack([c_f, c_b], axis=1)
    pos_agg = (state_h @ np.asarray(W1) + np.asarray(b1)).reshape(B, 1, 8)
    return output, state_h, state_c, pos_agg


# revision 28
# speedup vs baseline: 1.0041x; 1.0015x over previous
"""Bidirectional masked LSTM (Keras semantics) on 8 Trainium2 NeuronCores.

Problem: B=128, T=512, D=3, UNITS=512, G=2048 per direction.
Sharding: 8 cores = 2 directions x 4 batch quarters (B=32/core), no cross-core
communication. Backward direction runs on time-reversed inputs prepared on host.

Per-core per-step device computation (z fold layout, all matmuls fp16 with
fp32 PSUM accumulation):
  - 4 PE col-groups (tile_position=(0,32g)): group g owns unit-chunk g
    (units 128g..128g+128). PSUM z tile [128, 512]: partition 32g+i =
    (unit-chunk g, batch i); free = [f|i|o|g] gate blocks of 128 (unit within
    chunk). Per group: 1 K=5 matmul (x, bias, mask-fold) + 4 K=128 matmuls
    (h.T stationary, Wh moving) accumulate z; the last K tile is split into
    two N=256 halves so the f,i half ungates sigmoid earlier.
  - Mask folded into gates: row 4 of the x-side stationary is (1-m_t) with
    weight row (+BIG for f, -BIG for i/o, 0 for g), so masked steps give
    i'=o'=0, f'=1 => c carries, and h = h_prev*(1-m) + o'*tanh(c).
  - EW: sigmoid(f,i) then tanh(g) then sigmoid(o) on ScalarE; DVE chain
    cf = f'*c, ab = i'*g, c = cf+ab, ot = o'*tanh(c), h = hm + ot where
    hm = h_prev*(1-m_t) is computed off the critical path.
  - PE transpose of h fold -> h.T stationary for next step; dummy filler
    matmuls keep the PE clock gate (HAM) at 2.4 GHz through the elementwise
    window.
"""

import sys

sys.path.insert(0, "/opt/trn_rl_repo")

import ml_dtypes
import numpy as np

import concourse.bacc as bacc
import concourse.bass as bass
import concourse.mybir as mybir
import concourse.tile as tile
from concourse.bass_utils import run_bass_kernel_spmd
from concourse.masks import make_identity

B, T, D, U = 128, 512, 3, 512
G = 4 * U
NCORES = 8
BQ = B // 4            # batch per core (32)
NG = 4                 # PE col groups
KT = U // 128          # K tiles (4)
BIG = 50.0             # mask logit fold
NDUMMY = 10            # PE filler matmuls per step (HAM warmth)
TRACE = False          # set True (with axon_prof imported) to profile
F32 = mybir.dt.float32
F32R = mybir.dt.float32r
BF16 = mybir.dt.float16  # 16-bit tensors use fp16: 10-bit mantissa, values all in [-60, 60]

# gate order within each group's 512 free columns: [f | i | o | g]
_SRC_GATE_OFF = [512, 0, 1536, 1024]  # Keras z layout: i,f,g,o


def _group_cols():
    """cols[g, c] = source Wh/Wx column for group g, free column c."""
    cols = np.empty((NG, 512), np.int64)
    ar = np.arange(128)
    for g in range(NG):
        for gi in range(4):
            cols[g, 128 * gi:128 * gi + 128] = _SRC_GATE_OFF[gi] + 128 * g + ar
    return cols


_COLS = _group_cols()


def _build_program(trace_label=None):
    nc = bacc.Bacc("TRN2", target_bir_lowering=False, debug=False,
                   enable_asserts=False, num_devices=NCORES)

    whc_d = nc.dram_tensor("whc", [128, KT * NG * 512], BF16, kind="ExternalInput").ap()
    wxc_d = nc.dram_tensor("wxc", [5, NG * 512], BF16, kind="ExternalInput").ap()
    xaug_d = nc.dram_tensor("xaug", [5, T * BQ], BF16, kind="ExternalInput").ap()
    minv_d = nc.dram_tensor("minv", [128, T], F32, kind="ExternalInput").ap()

    ys_d = nc.dram_tensor("ys", [T, 128, 128], BF16, kind="ExternalOutput").ap()
    hfin_d = nc.dram_tensor("hfin", [128, 128], BF16, kind="ExternalOutput").ap()
    cfin_d = nc.dram_tensor("cfin", [128, 128], F32, kind="ExternalOutput").ap()

    with tile.TileContext(nc) as tc:
        with (
            tc.tile_pool(name="const", bufs=1) as cpool,
            tc.tile_pool(name="state", bufs=3) as spool,
            tc.tile_pool(name="work", bufs=4) as wpool,
            tc.tile_pool(name="zps", bufs=4, space="PSUM") as zpool,
            tc.tile_pool(name="trps", bufs=2, space="PSUM") as tpool,
            tc.tile_pool(name="wps", bufs=1, space="PSUM") as wupool,
        ):
            whc = cpool.tile([128, KT * NG * 512], BF16)
            wxc = cpool.tile([5, NG * 512], BF16)
            xaug = cpool.tile([5, T * BQ], BF16)
            minv = cpool.tile([128, T], F32)
            ident = cpool.tile([128, 128], BF16)

            nc.sync.dma_start(whc[:], whc_d[:])
            nc.sync.dma_start(wxc[:], wxc_d[:])
            nc.sync.dma_start(xaug[:], xaug_d[:])
            nc.sync.dma_start(minv[:], minv_d[:])
            make_identity(nc, ident[:])

            h_prev = spool.tile([128, 128], BF16, tag="h", bufs=5)
            c_prev = spool.tile([128, 128], F32, tag="c")
            hT = spool.tile([128, 128], BF16, tag="hT", bufs=4)
            nc.vector.memset(h_prev[:], 0.0)
            nc.vector.memset(c_prev[:], 0.0)
            nc.vector.memset(hT[:], 0.0)

            # PE warmup: ~20 back-to-back matmuls (~8us) flips the HAM clock
            # gate to 2.4 GHz; steady-state PE gaps stay under the ~3.4us MID
            # window so it never re-throttles.
            wups = wupool.tile([128, 512], F32)
            for _ in range(20):
                nc.tensor.matmul(wups[:], whc[:, 0:128], whc[:, 0:512],
                                 start=True, stop=True)

            for t in range(T):
                z = zpool.tile([128, 512], F32, tag="z")
                # x-side matmuls first: no dependency on h, can run during
                # previous step's elementwise chain.
                xs = xaug[:, t * BQ:(t + 1) * BQ]
                for g in range(NG):
                    nc.tensor.matmul(
                        z[32 * g:32 * g + 32, :], xs,
                        wxc[:, g * 512:(g + 1) * 512],
                        start=True, stop=False, tile_position=(0, 32 * g),
                    )
                for k in range(KT - 1):
                    hs = hT[:, 32 * k:32 * k + 32]
                    for g in range(NG):
                        nc.tensor.matmul(
                            z[32 * g:32 * g + 32, :], hs,
                            whc[:, (k * NG + g) * 512:(k * NG + g + 1) * 512],
                            start=False, stop=False,
                            tile_position=(0, 32 * g),
                        )
                # last K tile split in half along free dim: the f,i half
                # completes first and ungates sigmoid earlier
                k = KT - 1
                hs = hT[:, 32 * k:32 * k + 32]
                for half in range(2):
                    fs = slice(half * 256, half * 256 + 256)
                    for g in range(NG):
                        nc.tensor.matmul(
                            z[32 * g:32 * g + 32, fs], hs,
                            whc[:, (k * NG + g) * 512 + half * 256:
                                (k * NG + g) * 512 + half * 256 + 256],
                            start=False, stop=True,
                            tile_position=(0, 32 * g),
                        )

                # filler matmuls: keep the PE HAM-busy through the elementwise
                # window so the clock gate stays at 2.4 GHz (idle re-throttles
                # to 1.2 GHz and the per-step bursts never re-warm it)
                for _ in range(NDUMMY):
                    nc.tensor.matmul(wups[:], whc[:, 0:128], whc[:, 0:512],
                                     start=True, stop=True)

                # mask-blend term: independent of z; low priority so the
                # scheduler cannot slot it between transpose and the hT copies
                hm = wpool.tile([128, 128], BF16, tag="hm")
                with tc.high_priority(offset=-20):
                    nc.vector.tensor_scalar_mul(hm[:], h_prev[:], minv[:, t:t + 1])

                gb = wpool.tile([128, 128], BF16, tag="gb")
                sbif = wpool.tile([128, 256], BF16, tag="sbif")
                sbo = wpool.tile([128, 128], BF16, tag="sbo")
                nc.scalar.activation(sbif[:], z[:, 0:256],
                                     mybir.ActivationFunctionType.Sigmoid)
                nc.scalar.activation(gb[:], z[:, 384:512],
                                     mybir.ActivationFunctionType.Tanh)
                nc.scalar.activation(sbo[:], z[:, 256:384],
                                     mybir.ActivationFunctionType.Sigmoid)

                ab = wpool.tile([128, 128], BF16, tag="ab")
                cf = wpool.tile([128, 128], F32, tag="cf")
                c_new = spool.tile([128, 128], F32, tag="c")
                nc.vector.tensor_mul(cf[:], sbif[:, 0:128], c_prev[:])
                nc.vector.tensor_mul(ab[:], sbif[:, 128:256], gb[:])
                nc.vector.tensor_add(c_new[:], cf[:], ab[:])

                th = wpool.tile([128, 128], BF16, tag="th")
                nc.scalar.activation(th[:], c_new[:],
                                     mybir.ActivationFunctionType.Tanh)

                ot = wpool.tile([128, 128], BF16, tag="ot")
                h_new = spool.tile([128, 128], BF16, tag="h", bufs=5)
                nc.vector.tensor_mul(ot[:], sbo[:], th[:])
                nc.vector.tensor_add(h_new[:], hm[:], ot[:])

                tr = tpool.tile([128, 128], BF16, tag="tr")
                hT = spool.tile([128, 128], BF16, tag="hT", bufs=4)
                nc.tensor.transpose(tr[:], h_new[:], ident[:])
                nc.vector.tensor_copy(hT[:, 0:32], tr[:, 0:32])
                nc.vector.tensor_copy(hT[:, 32:128], tr[:, 32:128])

                nc.sync.dma_start(ys_d[t], hT[:])

                h_prev, c_prev = h_new, c_new

            nc.sync.dma_start(hfin_d[:], h_prev[:])
            nc.sync.dma_start(cfin_d[:], c_prev[:])

    nc.compile()
    return nc


_PROGRAM = None


def _program():
    global _PROGRAM
    if _PROGRAM is None:
        _PROGRAM = _build_program()
    return _PROGRAM


def _prep_core(seq, msk, Wx, Wh, b):
    """Per-core input arrays. seq [BQ,T,D] f32, msk [BQ,T] bool (already
    time-reversed for backward cores)."""
    whc = np.empty((128, KT * NG * 512), np.float16)
    for k in range(KT):
        wk = Wh[128 * k:128 * (k + 1)]
        for g in range(NG):
            whc[:, (k * NG + g) * 512:(k * NG + g + 1) * 512] = wk[:, _COLS[g]]

    mrow = np.zeros(G, np.float32)
    mrow[0:512] = -BIG          # i
    mrow[512:1024] = BIG        # f
    mrow[1536:2048] = -BIG      # o
    wxc = np.empty((5, NG * 512), np.float16)
    for g in range(NG):
        blk = slice(g * 512, (g + 1) * 512)
        wxc[0:3, blk] = Wx[:, _COLS[g]]
        wxc[3, blk] = b[_COLS[g]]
        wxc[4, blk] = mrow[_COLS[g]]

    inv = (1.0 - msk.astype(np.float32))            # [BQ, T]
    xaug = np.empty((5, T * BQ), np.float16)
    xaug[0:3] = seq.transpose(2, 1, 0).reshape(3, T * BQ)   # [d, t, i]
    xaug[3] = 1.0
    xaug[4] = inv.T.reshape(T * BQ)                  # [t, i]
    minv = np.tile(inv[None], (4, 1, 1)).reshape(128, T)    # [32j+i, t]
    return {"whc": whc, "wxc": wxc, "xaug": xaug, "minv": minv}


def _unfold_seq(ys):
    """[T,128,128] transposed fold (ys[t, u, 32j+i] = h[i, 128j+u]) -> [BQ, T, U]."""
    a = np.asarray(ys, np.float32).reshape(T, 128, 4, BQ)
    return a.transpose(3, 0, 2, 1).reshape(BQ, T, U)


def _unfold_state(s):
    """[128,128] fold -> [BQ, U]."""
    return np.asarray(s, np.float32).reshape(4, BQ, 128).transpose(1, 0, 2).reshape(BQ, U)


def kernel(sequence, mask, Wx_f, Wh_f, b_f, Wx_b, Wh_b, b_b, W1, b1, **extra):
    sequence = np.asarray(sequence, np.float32)
    mask = np.asarray(mask)
    nc = _program()

    in_maps = []
    for core in range(NCORES):
        d, q = divmod(core, 4)
        sl = slice(q * BQ, (q + 1) * BQ)
        seq, msk = sequence[sl], mask[sl]
        if d == 1:
            seq, msk = seq[:, ::-1], msk[:, ::-1]
        if d == 0:
            in_maps.append(_prep_core(seq, msk, np.asarray(Wx_f), np.asarray(Wh_f), np.asarray(b_f)))
        else:
            in_maps.append(_prep_core(seq, msk, np.asarray(Wx_b), np.asarray(Wh_b), np.asarray(b_b)))

    res = None
    for attempt in range(3):
        try:
            res = run_bass_kernel_spmd(nc, in_maps, list(range(NCORES)), trace=TRACE)
            break
        except Exception:
            # transient NRT_EXEC_UNIT_UNRECOVERABLE device wedges recover on
            # re-execution; re-raise only if persistent
            if attempt == 2:
                raise
    if TRACE and res.exec_time_ns:
        print(f"HW exec time: {res.exec_time_ns} ns")

    out_f = np.empty((B, T, U), np.float32)
    out_b = np.empty((B, T, U), np.float32)
    h_f = np.empty((B, U), np.float32)
    h_b = np.empty((B, U), np.float32)
    c_f = np.empty((B, U), np.float32)
    c_b = np.empty((B, U), np.float32)
    for core in range(NCORES):
        d, q = divmod(core, 4)
        sl = slice(q * BQ, (q + 1) * BQ)
        r = res.results[core]
        ys = _unfold_seq(r["ys"])
        if d == 0:
            out_f[sl] = ys
            h_f[sl] = _unfold_state(r["hfin"])
            c_f[sl] = _unfold_state(r["cfin"])
        else:
            out_b[sl] = ys[:, ::-1]
            h_b[sl] = _unfold_state(r["hfin"])
            c_b[sl] = _unfold_state(r["cfin"])

    output = np.concatenate([out_f, out_b], axis=-1)
    state_h = np.stack([h_f, h_b], axis=1)
    state_c = np.stack([c_f, c_b], axis=1)
    pos_agg = (state_h @ np.asarray(W1) + np.asarray(b1)).reshape(B, 1, 8)
    return output, state_h, state_c, pos_agg
